# revision 32
# baseline (speedup 1.0000x reference)
"""MoE gate (router) kernel for Trainium2, data-parallel over 8 NeuronCores.

Reference computation (per problem nn_MoEGate):
    x = hidden_states.reshape(-1, H)          # [T=32768, H=1024]
    logits = x @ weight.T                     # [T, E=64]
    scores = softmax(logits)
    topk_weight, topk_idx = top_k(scores, 2); topk_weight /= sum(topk_weight)
    aux_loss = seq-aux loss over per-batch expert counts and mean scores

Sharding: tokens are split 4096 per core (each core = exactly one batch
element of the [8, 4096, H] input), router weight replicated. Each core
returns its top-2 idx/weights plus the per-expert sum of softmax scores;
the tiny aux-loss reduction is finished on the host in float64.

Device pipeline per core (T_c = 4096 tokens = 8 groups x 512):
  - DMA x tiles [128, 1024] fp32 (natural layout)
  - PE transposes x -> xT chunks [h=128, t=512] staged via PSUM -> SBUF
  - fp32 matmul (col-tiled 2x: h-chunks 0-3 -> PSUM cols 0-63, 4-7 ->
    64-127) computes logits.T [64, 512]; halves summed during PSUM
    evacuation; small PE transposes give logits tiles [128 tok, 64 exp]
  - DVE max8/max_index give the top-8 values/indices per token (exact
    jax.lax.top_k tie semantics); top-2 sliced out on host
  - ACT exp(l - max) with accumulated row-sum Z; Pool scales by 1/Z; a
    ones-vector bf16 matmul accumulates sum-of-softmax per expert
  - normalized top-2 weights: w1 = 1/(1+exp(v2-v1)), w2 = 1-w1
"""
import sys

sys.path.insert(0, "/opt/trn_rl_repo")

import numpy as np

TOP_K = 2
ALPHA = 0.1
B, S, H, E = 8, 4096, 1024, 64
N_CORES = 8
T_C = S  # tokens per core (= one batch element)
N_TILES = T_C // 128       # 32
N_GROUPS = N_TILES // 4    # 8


def _build_kernel():
    import concourse.bacc as bacc
    import concourse.mybir as mybir
    import concourse.tile as tile
    
    F32 = mybir.dt.float32
    BF16 = mybir.dt.bfloat16
    U32 = mybir.dt.uint32
    AF = mybir.ActivationFunctionType

    nc = bacc.Bacc("TRN2", target_bir_lowering=False, debug=False)

    x_d = nc.dram_tensor("x", [T_C, H], F32, kind="ExternalInput").ap()
    wt_d = nc.dram_tensor("wt", [H, E], F32, kind="ExternalInput").ap()
    eye_d = nc.dram_tensor("eye", [128, 128], F32, kind="ExternalInput").ap()
    i8_d = nc.dram_tensor("i8", [128, N_TILES, 8], U32, kind="ExternalOutput").ap()
    w2_d = nc.dram_tensor("w2", [128, N_TILES, 2], F32, kind="ExternalOutput").ap()
    ms_d = nc.dram_tensor("ms", [1, 256], F32, kind="ExternalOutput").ap()

    with tile.TileContext(nc) as tc:
        with tc.tile_pool(name="cst", bufs=1) as cpool, \
             tc.tile_pool(name="xp", bufs=16) as xpool, \
             tc.tile_pool(name="xtg", bufs=2) as xtpool, \
             tc.tile_pool(name="lg", bufs=2) as lgpool, \
             tc.tile_pool(name="st", bufs=4) as stpool, \
             tc.tile_pool(name="sm", bufs=4) as smpool, \
             tc.tile_pool(name="pt", bufs=3, space="PSUM") as pt_ps, \
             tc.tile_pool(name="lp", bufs=2, space="PSUM") as lp_ps, \
             tc.tile_pool(name="lt", bufs=2, space="PSUM") as lt_ps, \
             tc.tile_pool(name="msp", bufs=1, space="PSUM") as ms_ps:

            ident = cpool.tile([128, 128], F32)
            nc.sync.dma_start(ident, eye_d)
            wt_sb = cpool.tile([128, 8, E], F32)

            ones_bf = cpool.tile([128, 1], BF16)
            nc.gpsimd.memset(ones_bf, 1.0)

            vbuf = cpool.tile([128, N_TILES, 8], F32)
            ibuf = cpool.tile([128, N_TILES, 8], U32)
            wbuf = cpool.tile([128, N_TILES, 2], F32)
            zbuf = cpool.tile([128, N_TILES], F32)

            ms_psum = ms_ps.tile([1, 256], F32)
            p_grps = []

            for g in range(N_GROUPS):
                g4 = g * 4
                xts = []
                for s in range(4):
                    xt = xpool.tile([128, H], F32, tag="xt")
                    nc.sync.dma_start(xt, x_d[(g4 + s) * 128:(g4 + s + 1) * 128, :])
                    xts.append(xt)
                if g == 0:
                    # router weight loads behind the first x tiles (needed ~6us later)
                    nc.sync.dma_start(wt_sb, wt_d.rearrange("(c p) e -> p c e", p=128))

                # transpose to xtg [h=128, chunk, t=512]
                xtg = xtpool.tile([128, 8, 512], F32, tag="xtg")
                # transposes, PSUM evacuation, and col-tiled fp32 GEMM,
                # interleaved per chunk to keep the PE dense
                lp = lp_ps.tile([128, 512], F32, tag="lp")
                for c in (0, 4, 1, 5, 2, 6, 3, 7):
                    pt = pt_ps.tile([128, 512], F32, tag="pt")
                    for s in range(4):
                        nc.tensor.transpose(pt[:, s * 128:(s + 1) * 128],
                                            xts[s][:, c * 128:(c + 1) * 128], ident)
                    with tc.high_priority():
                        if c % 2 == 0:
                            nc.vector.tensor_copy(xtg[:, c], pt)
                        else:
                            nc.scalar.activation(xtg[:, c], pt, AF.Copy)
                    if c == 1 and g > 0:
                        # previous group's softmax-sum accumulation (operands
                        # long since ready -> no PE stall)
                        nc.tensor.matmul(ms_psum, lhsT=ones_bf,
                                         rhs=p_grps[g - 1].rearrange("p s e -> p (s e)"),
                                         start=(g == 1), stop=False)
                    if c < 4:
                        nc.tensor.matmul(lp[0:64, :], lhsT=wt_sb[:, c], rhs=xtg[:, c],
                                         start=(c == 0), stop=(c == 3),
                                         tile_position=(0, 0))
                    else:
                        nc.tensor.matmul(lp[64:128, :], lhsT=wt_sb[:, c], rhs=xtg[:, c],
                                         start=(c == 4), stop=(c == 7),
                                         tile_position=(0, 64))

                half = lgpool.tile([64, 512], F32, tag="half")
                lg_sb = lgpool.tile([64, 512], F32, tag="lg_sb")
                with tc.high_priority():
                    nc.scalar.activation(half, lp[0:64, :], AF.Copy)
                    nc.vector.tensor_add(lg_sb, half, lp[64:128, :])

                # transpose logits to [token, expert] tiles
                ltp = lt_ps.tile([128, 256], F32, tag="ltp")
                for s in range(4):
                    nc.tensor.transpose(ltp[:, s * 64:(s + 1) * 64],
                                        lg_sb[:, s * 128:(s + 1) * 128],
                                        ident[0:64, 0:64])

                # top-8 values + indices per token tile
                for s in range(4):
                    i = g4 + s
                    lgt = ltp[:, s * 64:(s + 1) * 64]
                    nc.vector.max(out=vbuf[:, i], in_=lgt)
                    nc.vector.max_index(out=ibuf[:, i], in_max=vbuf[:, i],
                                        in_values=lgt)

                negm4 = smpool.tile([128, 4], F32, tag="negm4")
                nc.gpsimd.tensor_scalar_mul(negm4, vbuf[:, g4:g4 + 4, 0], -1.0)

                st_grp = stpool.tile([128, 4, 64], F32, tag="st_grp")
                for s in range(4):
                    i = g4 + s
                    nc.scalar.activation(st_grp[:, s], ltp[:, s * 64:(s + 1) * 64],
                                         AF.Exp, bias=negm4[:, s:s + 1], scale=1.0,
                                         accum_out=zbuf[:, i:i + 1])

                rz4 = smpool.tile([128, 4], F32, tag="rz4")
                nc.vector.reciprocal(rz4, zbuf[:, g4:g4 + 4])
                p_grp = stpool.tile([128, 4, 64], BF16, tag="p_grp")
                nc.vector.tensor_mul(p_grp, st_grp,
                                     rz4[:, :, None].to_broadcast([128, 4, 64]))
                p_grps.append(p_grp)

                # normalized top-2 weights for the group
                d4 = smpool.tile([128, 4], F32, tag="d4")
                nc.gpsimd.tensor_sub(d4, vbuf[:, g4:g4 + 4, 1], vbuf[:, g4:g4 + 4, 0])
                s2 = smpool.tile([128, 4], F32, tag="s2")
                nc.scalar.activation(s2, d4, AF.Exp)
                den = smpool.tile([128, 4], F32, tag="den")
                nc.gpsimd.tensor_scalar_add(den, s2, 1.0)
                nc.vector.reciprocal(wbuf[:, g4:g4 + 4, 0], den)
                nc.gpsimd.tensor_mul(wbuf[:, g4:g4 + 4, 1], s2, wbuf[:, g4:g4 + 4, 0])

                nc.sync.dma_start(i8_d[:, g4:g4 + 4, :], ibuf[:, g4:g4 + 4, :])
                nc.sync.dma_start(w2_d[:, g4:g4 + 4, :], wbuf[:, g4:g4 + 4, :])

            nc.tensor.matmul(ms_psum, lhsT=ones_bf,
                             rhs=p_grps[N_GROUPS - 1].rearrange("p s e -> p (s e)"),
                             start=False, stop=True)
            ms_sb = cpool.tile([1, 256], F32)
            nc.vector.tensor_copy(ms_sb, ms_psum)
            nc.sync.dma_start(ms_d, ms_sb)

    nc.compile()
    return nc


_NC_CACHE = None


def kernel(hidden_states: np.ndarray, weight: np.ndarray):
    global _NC_CACHE
    from concourse.bass_utils import run_bass_kernel_spmd

    if _NC_CACHE is None:
        _NC_CACHE = _build_kernel()
    nc = _NC_CACHE

    x = np.ascontiguousarray(hidden_states.reshape(B * S, H), dtype=np.float32)
    wt = np.ascontiguousarray(weight.T, dtype=np.float32)

    eye = np.eye(128, dtype=np.float32)
    in_maps = [{"x": x[c * T_C:(c + 1) * T_C], "wt": wt, "eye": eye}
               for c in range(N_CORES)]
    res = run_bass_kernel_spmd(nc, in_maps, core_ids=list(range(N_CORES)))

    topk_idx = np.empty((B * S, TOP_K), np.int32)
    topk_weight = np.empty((B * S, TOP_K), np.float32)
    aux_terms = np.empty(B, np.float64)

    for c in range(N_CORES):
        r = res.results[c]
        idx_c = r["i8"][:, :, :TOP_K].astype(np.int64)          # [128, 32, 2]
        w_c = r["w2"]                                           # [128, 32, 2]
        topk_idx[c * T_C:(c + 1) * T_C] = np.transpose(
            idx_c, (1, 0, 2)).reshape(T_C, TOP_K).astype(np.int32)
        topk_weight[c * T_C:(c + 1) * T_C] = np.transpose(
            w_c, (1, 0, 2)).reshape(T_C, TOP_K)
        # per-expert sum of softmax scores over this core's S tokens
        score_sum = r["ms"].reshape(4, E).astype(np.float64).sum(0)   # [64]
        counts = np.bincount(idx_c[:, :, :TOP_K].reshape(-1), minlength=E
                             ).astype(np.float64)
        ce = counts / (S * TOP_K / E)
        mean_scores = score_sum / S
        aux_terms[c] = np.dot(ce, mean_scores)

    aux_loss = np.float32(aux_terms.mean() * ALPHA)
    return topk_idx, topk_weight, aux_loss


# revision 33
# speedup vs baseline: 1.0294x; 1.0294x over previous
"""MoE gate (router) kernel for Trainium2, data-parallel over 8 NeuronCores.

Reference computation (per problem nn_MoEGate):
    x = hidden_states.reshape(-1, H)          # [T=32768, H=1024]
    logits = x @ weight.T                     # [T, E=64]
    scores = softmax(logits)
    topk_weight, topk_idx = top_k(scores, 2); topk_weight /= sum(topk_weight)
    aux_loss = seq-aux loss over per-batch expert counts and mean scores

Sharding: tokens are split 4096 per core (each core = exactly one batch
element of the [8, 4096, H] input), router weight replicated. Each core
returns its top-2 idx/weights plus the per-expert sum of softmax scores;
the tiny aux-loss reduction is finished on the host in float64.

Device pipeline per core (T_c = 4096 tokens = 8 groups x 512):
  - DMA x tiles [128, 1024] fp32 (natural layout)
  - PE transposes x -> xT chunks [h=128, t=512] staged via PSUM -> SBUF
  - fp32 matmul (col-tiled 2x: h-chunks 0-3 -> PSUM cols 0-63, 4-7 ->
    64-127) computes logits.T [64, 512]; halves summed during PSUM
    evacuation; small PE transposes give logits tiles [128 tok, 64 exp]
  - DVE max8/max_index give the top-8 values/indices per token (exact
    jax.lax.top_k tie semantics); top-2 sliced out on host
  - ACT exp(l - max) with accumulated row-sum Z; Pool scales by 1/Z; a
    ones-vector bf16 matmul accumulates sum-of-softmax per expert
  - normalized top-2 weights: w1 = 1/(1+exp(v2-v1)), w2 = 1-w1
"""
import sys

sys.path.insert(0, "/opt/trn_rl_repo")

import numpy as np

TOP_K = 2
ALPHA = 0.1
B, S, H, E = 8, 4096, 1024, 64
N_CORES = 8
T_C = S  # tokens per core (= one batch element)
N_TILES = T_C // 128       # 32
N_GROUPS = N_TILES // 4    # 8


def _build_kernel():
    import concourse.bacc as bacc
    import concourse.mybir as mybir
    import concourse.tile as tile
    
    F32 = mybir.dt.float32
    BF16 = mybir.dt.bfloat16
    U32 = mybir.dt.uint32
    AF = mybir.ActivationFunctionType

    nc = bacc.Bacc("TRN2", target_bir_lowering=False, debug=False)

    x_d = nc.dram_tensor("x", [T_C, H], F32, kind="ExternalInput").ap()
    wt_d = nc.dram_tensor("wt", [H, E], F32, kind="ExternalInput").ap()
    eye_d = nc.dram_tensor("eye", [128, 128], F32, kind="ExternalInput").ap()
    i8_d = nc.dram_tensor("i8", [128, N_TILES, 8], U32, kind="ExternalOutput").ap()
    w2_d = nc.dram_tensor("w2", [128, N_TILES, 2], F32, kind="ExternalOutput").ap()
    ms_d = nc.dram_tensor("ms", [1, 256], F32, kind="ExternalOutput").ap()

    with tile.TileContext(nc) as tc:
        with tc.tile_pool(name="cst", bufs=1) as cpool, \
             tc.tile_pool(name="xp", bufs=16) as xpool, \
             tc.tile_pool(name="xtg", bufs=2) as xtpool, \
             tc.tile_pool(name="lg", bufs=2) as lgpool, \
             tc.tile_pool(name="st", bufs=4) as stpool, \
             tc.tile_pool(name="sm", bufs=4) as smpool, \
             tc.tile_pool(name="pt", bufs=3, space="PSUM") as pt_ps, \
             tc.tile_pool(name="lp", bufs=2, space="PSUM") as lp_ps, \
             tc.tile_pool(name="lt", bufs=2, space="PSUM") as lt_ps, \
             tc.tile_pool(name="msp", bufs=1, space="PSUM") as ms_ps:

            ident = cpool.tile([128, 128], F32)
            nc.sync.dma_start(ident, eye_d)
            wt_sb = cpool.tile([128, 8, E], F32)

            # HAM warm-up: dummy matmuls on scratch SBUF while the first x
            # tiles are still in flight (PE would otherwise idle ~12us cold)
            warm_a = cpool.tile([128, 64], F32)
            nc.gpsimd.memset(warm_a, 0.0)
            warm_b = cpool.tile([128, 512], F32)
            nc.gpsimd.memset(warm_b, 0.0)
            warm_ps = lt_ps.tile([64, 512], F32, tag="ltp")
            for _ in range(4):
                nc.tensor.matmul(warm_ps, lhsT=warm_a, rhs=warm_b,
                                 start=True, stop=True, skip_group_check=True)
            ones_bf = cpool.tile([128, 1], BF16)
            nc.gpsimd.memset(ones_bf, 1.0)

            vbuf = cpool.tile([128, N_TILES, 8], F32)
            ibuf = cpool.tile([128, N_TILES, 8], U32)
            wbuf = cpool.tile([128, N_TILES, 2], F32)
            zbuf = cpool.tile([128, N_TILES], F32)

            ms_psum = ms_ps.tile([1, 256], F32)
            p_grps = []

            for g in range(N_GROUPS):
                g4 = g * 4
                xts = []
                for s in range(4):
                    xt = xpool.tile([128, H], F32, tag="xt")
                    nc.sync.dma_start(xt, x_d[(g4 + s) * 128:(g4 + s + 1) * 128, :])
                    xts.append(xt)
                if g == 0:
                    # router weight loads behind the first x tiles (needed ~6us later)
                    nc.sync.dma_start(wt_sb, wt_d.rearrange("(c p) e -> p c e", p=128))

                # transpose to xtg [h=128, chunk, t=512]
                xtg = xtpool.tile([128, 8, 512], F32, tag="xtg")
                # transposes, PSUM evacuation, and col-tiled fp32 GEMM,
                # interleaved per chunk to keep the PE dense
                lp = lp_ps.tile([128, 512], F32, tag="lp")
                for c in (0, 4, 1, 5, 2, 6, 3, 7):
                    pt = pt_ps.tile([128, 512], F32, tag="pt")
                    for s in range(4):
                        nc.tensor.transpose(pt[:, s * 128:(s + 1) * 128],
                                            xts[s][:, c * 128:(c + 1) * 128], ident)
                    with tc.high_priority():
                        if c % 2 == 0:
                            nc.vector.tensor_copy(xtg[:, c], pt)
                        else:
                            nc.scalar.activation(xtg[:, c], pt, AF.Copy)
                    if c == 1 and g > 0:
                        # previous group's softmax-sum accumulation (operands
                        # long since ready -> no PE stall)
                        nc.tensor.matmul(ms_psum, lhsT=ones_bf,
                                         rhs=p_grps[g - 1].rearrange("p s e -> p (s e)"),
                                         start=(g == 1), stop=False)
                    if c < 4:
                        nc.tensor.matmul(lp[0:64, :], lhsT=wt_sb[:, c], rhs=xtg[:, c],
                                         start=(c == 0), stop=(c == 3),
                                         tile_position=(0, 0))
                    else:
                        nc.tensor.matmul(lp[64:128, :], lhsT=wt_sb[:, c], rhs=xtg[:, c],
                                         start=(c == 4), stop=(c == 7),
                                         tile_position=(0, 64))

                half = lgpool.tile([64, 512], F32, tag="half")
                lg_sb = lgpool.tile([64, 512], F32, tag="lg_sb")
                with tc.high_priority():
                    nc.scalar.activation(half, lp[0:64, :], AF.Copy)
                    nc.vector.tensor_add(lg_sb, half, lp[64:128, :])

                # transpose logits to [token, expert] tiles
                ltp = lt_ps.tile([128, 256], F32, tag="ltp")
                for s in range(4):
                    nc.tensor.transpose(ltp[:, s * 64:(s + 1) * 64],
                                        lg_sb[:, s * 128:(s + 1) * 128],
                                        ident[0:64, 0:64])

                # top-8 values + indices per token tile
                for s in range(4):
                    i = g4 + s
                    lgt = ltp[:, s * 64:(s + 1) * 64]
                    nc.vector.max(out=vbuf[:, i], in_=lgt)
                    nc.vector.max_index(out=ibuf[:, i], in_max=vbuf[:, i],
                                        in_values=lgt)

                negm4 = smpool.tile([128, 4], F32, tag="negm4")
                nc.gpsimd.tensor_scalar_mul(negm4, vbuf[:, g4:g4 + 4, 0], -1.0)

                st_grp = stpool.tile([128, 4, 64], F32, tag="st_grp")
                for s in range(4):
                    i = g4 + s
                    nc.scalar.activation(st_grp[:, s], ltp[:, s * 64:(s + 1) * 64],
                                         AF.Exp, bias=negm4[:, s:s + 1], scale=1.0,
                                         accum_out=zbuf[:, i:i + 1])

                rz4 = smpool.tile([128, 4], F32, tag="rz4")
                nc.vector.reciprocal(rz4, zbuf[:, g4:g4 + 4])
                p_grp = stpool.tile([128, 4, 64], BF16, tag="p_grp")
                nc.vector.tensor_mul(p_grp, st_grp,
                                     rz4[:, :, None].to_broadcast([128, 4, 64]))
                p_grps.append(p_grp)

                # normalized top-2 weights for the group
                d4 = smpool.tile([128, 4], F32, tag="d4")
                nc.gpsimd.tensor_sub(d4, vbuf[:, g4:g4 + 4, 1], vbuf[:, g4:g4 + 4, 0])
                s2 = smpool.tile([128, 4], F32, tag="s2")
                nc.scalar.activation(s2, d4, AF.Exp)
                den = smpool.tile([128, 4], F32, tag="den")
                nc.gpsimd.tensor_scalar_add(den, s2, 1.0)
                nc.vector.reciprocal(wbuf[:, g4:g4 + 4, 0], den)
                nc.gpsimd.tensor_mul(wbuf[:, g4:g4 + 4, 1], s2, wbuf[:, g4:g4 + 4, 0])

                nc.sync.dma_start(i8_d[:, g4:g4 + 4, :], ibuf[:, g4:g4 + 4, :])
                nc.sync.dma_start(w2_d[:, g4:g4 + 4, :], wbuf[:, g4:g4 + 4, :])

            nc.tensor.matmul(ms_psum, lhsT=ones_bf,
                             rhs=p_grps[N_GROUPS - 1].rearrange("p s e -> p (s e)"),
                             start=False, stop=True)
            ms_sb = cpool.tile([1, 256], F32)
            nc.vector.tensor_copy(ms_sb, ms_psum)
            nc.sync.dma_start(ms_d, ms_sb)

    nc.compile()
    return nc


_NC_CACHE = None


def kernel(hidden_states: np.ndarray, weight: np.ndarray):
    global _NC_CACHE
    from concourse.bass_utils import run_bass_kernel_spmd

    if _NC_CACHE is None:
        _NC_CACHE = _build_kernel()
    nc = _NC_CACHE

    x = np.ascontiguousarray(hidden_states.reshape(B * S, H), dtype=np.float32)
    wt = np.ascontiguousarray(weight.T, dtype=np.float32)

    eye = np.eye(128, dtype=np.float32)
    in_maps = [{"x": x[c * T_C:(c + 1) * T_C], "wt": wt, "eye": eye}
               for c in range(N_CORES)]
    res = run_bass_kernel_spmd(nc, in_maps, core_ids=list(range(N_CORES)))

    topk_idx = np.empty((B * S, TOP_K), np.int32)
    topk_weight = np.empty((B * S, TOP_K), np.float32)
    aux_terms = np.empty(B, np.float64)

    for c in range(N_CORES):
        r = res.results[c]
        idx_c = r["i8"][:, :, :TOP_K].astype(np.int64)          # [128, 32, 2]
        w_c = r["w2"]                                           # [128, 32, 2]
        topk_idx[c * T_C:(c + 1) * T_C] = np.transpose(
            idx_c, (1, 0, 2)).reshape(T_C, TOP_K).astype(np.int32)
        topk_weight[c * T_C:(c + 1) * T_C] = np.transpose(
            w_c, (1, 0, 2)).reshape(T_C, TOP_K)
        # per-expert sum of softmax scores over this core's S tokens
        score_sum = r["ms"].reshape(4, E).astype(np.float64).sum(0)   # [64]
        counts = np.bincount(idx_c[:, :, :TOP_K].reshape(-1), minlength=E
                             ).astype(np.float64)
        ce = counts / (S * TOP_K / E)
        mean_scores = score_sum / S
        aux_terms[c] = np.dot(ce, mean_scores)

    aux_loss = np.float32(aux_terms.mean() * ALPHA)
    return topk_idx, topk_weight, aux_loss
